# revision 1
# baseline (speedup 1.0000x reference)
"""Trainium2 Bass kernel for nn_MultiHeadHighLevelAllocator.

Math (reference):
    ue = MLP3(uav_feat)                            # (B,U,E)
    te = MLP3(task_feat)                           # (B,T,E)
    q  = ue[:,None,:,:] + head_q[None,:,None,:]    # (B,H,U,E)
    logits[b,h,u,t] = relu(q[b,h,u]@Wq + te[b,t]@Wk + fb1) @ fw2 + fb2

Key decomposition: by linearity of the projections,
    pre[b,h,u,t,:] = base[b,u,t,:] + hqP[h,:]
where base[b,u,t,:] = ue[b,u]@Wq + te[b,t]@Wk  (outer sum, H-independent)
and   hqP[h,:] = head_q[h]@Wq + fb1.

Per-core (data parallel over B, 2 batches/core):
  1. Encoders on TensorE in transposed layout (feat x rows), ScalarE ReLU+bias.
  2. base tiles (128d x 512) produced by two accumulating matmuls into PSUM
     (stride-0 broadcast APs replicate ue columns over t / te block over u).
  3. Per head: ReLU(base + hqP[h]) with per-partition bias -> fp16 tiles
     (VectorE tensor_scalar add+max for 2 heads, ScalarE activation for 2).
  4. Reduction against fw2 via masked-stationary matmuls: a (128x32) fp16
     stationary holding fw2-chunk in column j writes the dot product row to
     PSUM partition 32g+j of strip-g's own bank, accumulating zeros elsewhere;
     16 u-blocks x 2 heads x 2 chunks accumulate per strip bank group.
  5. One (128x512) fp32 result tile (+fb2) DMAed out per core.

All per-core inputs are packed host-side into a single (128, 3727) fp32
tensor loaded by ONE DMA (PE instructions only support a single sync wait,
so first-use deps must collapse to one semaphore).
"""
import os
import sys

for _p in ("/opt/trn_rl_repo", "/root/.axon_site/_ro/trn_rl_repo"):
    if os.path.isdir(_p) and _p not in sys.path:
        sys.path.insert(0, _p)

import numpy as np
import concourse.bass as bass
import concourse.mybir as mybir
from concourse import tile

B, U, T = 16, 64, 128
UAV_DIM, TASK_DIM = 32, 32
E, H, HID = 128, 4, 256
ENC_H = 128
NCORES = 8
BL = B // NCORES          # batches per core
NBLK = U // 4             # 16 u-blocks of 4 us -> N=512 columns each
f32, f16 = mybir.dt.float32, mybir.dt.float16
bf16 = mybir.dt.bfloat16
AF = mybir.ActivationFunctionType
ALU = mybir.AluOpType

# packed constant-tensor column layout (fp32 columns)
_C_UAVT = 0          # (32, 128)
_C_TASKT = 128       # (32, 256)
_C_UW0 = 384         # (32, 128)
_C_TW0 = 512         # (32, 128)
_C_UW1 = 640         # (128, 128)
_C_UW2 = 768
_C_TW1 = 896
_C_TW2 = 1024
_C_ENCB = 1152       # (128, 7): ub0 ub1 ub2 tb0 tb1 tb2 fb2
_C_HQPB = 1159       # (128, 8): col c*4+h
_C_WQK = 1167        # (128, 512): Wq c0 | Wq c1 | Wk c0 | Wk c1
# wz: two 63-col fp16 segments; fw2 chunk c at col c*63+31. The (128x32)
# masked stationary with fw2 at column j is the window [c*63+31-j, +32).
_C_WZ = 1679
_C_TOTAL = 1805

_BUILD_PAT = ["G", "D", "A", "G", "D", "A", "G", "A",
              "G", "D", "A", "G", "G", "D", "A", "G"]

_cache: dict = {}


def _split_multi_waits(nc):
    """Walrus in this toolchain rejects >1 sync wait per engine instruction
    ("Too many sync wait commands"). Hoist extra waits onto preceding
    same-engine NoOps — identical semantics on the in-order engine queues."""
    n_split = 0
    for func in nc.m.functions:
        for bb in func.blocks:
            new = []
            for ins in bb.instructions:
                si = ins.sync_info
                waits = list(si.on_wait) if (si and si.on_wait) else []
                if len(waits) > 1:
                    for k, w in enumerate(waits[:-1]):
                        nop = mybir.InstNoOp(name=f"{ins.name}_hw{k}", ins=[], outs=[])
                        nop.engine = ins.engine
                        nop.sync_info = mybir.SyncInfo(on_wait=[w], on_update=[])
                        new.append(nop)
                        n_split += 1
                    si.on_wait = [waits[-1]]
                new.append(ins)
            bb.instructions = new
    return n_split


def _build_nc():
    nc = bass.Bass()
    packed = nc.dram_tensor("packed", [128, _C_TOTAL], f32, kind="ExternalInput")
    out = nc.dram_tensor("out", [128, 512], f32, kind="ExternalOutput")

    with tile.TileContext(nc) as tc:
        with (
            tc.tile_pool(name="const", bufs=1) as constp,
            tc.tile_pool(name="persist", bufs=1) as persistp,
            tc.tile_pool(name="encw", bufs=2) as encwp,
        ):
            A = constp.tile([128, _C_TOTAL], f32, tag="all")
            # phase-ordered loads so the encoders start as early as possible
            nc.sync.dma_start(A[:, :640], packed[:, :640])           # L1 inputs
            nc.sync.dma_start(A[:, 1152:_C_WQK], packed[:, 1152:_C_WQK])  # biases
            nc.sync.dma_start(A[:, 640:1152], packed[:, 640:1152])   # L2/L3 w
            nc.sync.dma_start(A[:, _C_WQK:], packed[:, _C_WQK:])     # wqk+wz
            # fp32 -> fp16 conversion on ScalarE: also serves as ScalarE's
            # first touch of the DMA'd tile, so later ACT instructions never
            # pair a DMA-sem wait with an engine-sem wait (ISA wait-slot
            # limits; PE matmuls only support a single wait).
            # wz holds fw2 chunk c in column c*63+31; the (128x32) masked
            # stationary with fw2 at column j is the window [c*63+31-j, +32).
            act_touch = constp.tile([128, 1], f32, tag="acttouch")
            nc.scalar.copy(act_touch[:], A[:, 0:1])
            sb_wz = constp.tile([128, 126], f16, tag="wz")
            nc.scalar.copy(sb_wz[:], A[:, _C_WZ:_C_WZ + 126])
            # VectorE first touch of the DMA'd tile (same wait-slot reason).
            dve_touch = constp.tile([128, 1], f32, tag="dvetouch")
            nc.vector.tensor_copy(dve_touch[:], A[:, 0:1])
            # fp16 projection weights: fp32 moving operands stream at half
            # rate through the PE array, so the base matmuls run fp16.
            sb_wqk16 = constp.tile([128, 512], f16, tag="wqk16")
            nc.scalar.copy(sb_wqk16[:], A[:, _C_WQK:_C_WQK + 512])

            enc_w = {
                "uw0": A[0:32, _C_UW0:_C_UW0 + 128],
                "tw0": A[0:32, _C_TW0:_C_TW0 + 128],
                "uw1": A[:, _C_UW1:_C_UW1 + 128],
                "uw2": A[:, _C_UW2:_C_UW2 + 128],
                "tw1": A[:, _C_TW1:_C_TW1 + 128],
                "tw2": A[:, _C_TW2:_C_TW2 + 128],
            }

            def encb_col(i):
                return A[:, _C_ENCB + i:_C_ENCB + i + 1]

            # ---- pools for the whole kernel (8 PSUM banks exactly:
            #      encoder 1 + base 3 + logits 4) ----
            with (
                tc.tile_pool(name="bsbp", bufs=3) as bsbp,
                tc.tile_pool(name="relup", bufs=6) as relup,
                tc.tile_pool(name="outp", bufs=1) as outp,
                tc.tile_pool(name="bpp", bufs=4, space="PSUM") as psB,
                tc.tile_pool(name="lpp", bufs=1, space="PSUM") as psL,
            ):
                # ---- encoders: ue/te chains interleaved so PE and
                #      ScalarE ping-pong instead of serializing ----
                chains = {
                    "ue": [A[0:32, _C_UAVT:_C_UAVT + BL * U], BL * U,
                           ("uw0", "uw1", "uw2"), (0, 1, 2)],
                    "te": [A[0:32, _C_TASKT:_C_TASKT + BL * T], BL * T,
                           ("tw0", "tw1", "tw2"), (3, 4, 5)],
                }
                cur = {k: v[0] for k, v in chains.items()}
                for li in range(3):
                    pss = {}
                    for k, (x0, rows, wn, bc) in chains.items():
                        ps = psB.tile([128, 512], f32, tag="bp",
                                      name=f"ps{k}{li}")
                        nc.tensor.matmul(ps[:, :rows], enc_w[wn[li]], cur[k],
                                         start=True, stop=True)
                        pss[k] = ps
                    for k, (x0, rows, wn, bc) in chains.items():
                        if li < 2:
                            nxt = encwp.tile([128, rows], f32, tag=f"{k}h",
                                             name=f"{k}h{li}")
                            nc.scalar.activation(nxt[:], pss[k][:, :rows],
                                                 AF.Relu,
                                                 bias=encb_col(bc[li]),
                                                 scale=1.0)
                        else:
                            nxt = persistp.tile([128, rows], f16, tag=f"{k}T",
                                                name=f"{k}T")
                            nc.scalar.activation(nxt[:], pss[k][:, :rows],
                                                 AF.Identity,
                                                 bias=encb_col(bc[li]),
                                                 scale=1.0)
                        cur[k] = nxt[:]
                ueT, teT = cur["ue"], cur["te"]

                # ---- all (b,c) section preludes upfront: khP/qP tiles ----
                khPs, qPs = {}, {}
                for b in range(BL):
                    for c in range(2):
                        pk = psB.tile([128, 512], f32, tag="bp",
                                      name=f"pk{b}{c}")
                        nc.tensor.matmul(pk[:, :T],
                                         sb_wqk16[:, 256 + c * 128:256 + (c + 1) * 128],
                                         teT[:, b * T:(b + 1) * T],
                                         start=True, stop=True)
                        pq = psB.tile([128, 512], f32, tag="bp",
                                      name=f"pq{b}{c}")
                        nc.tensor.matmul(pq[:, :U],
                                         sb_wqk16[:, c * 128:(c + 1) * 128],
                                         ueT[:, b * U:(b + 1) * U],
                                         start=True, stop=True)
                        khP = persistp.tile([128, T], f16, tag=f"khP{b}{c}",
                                            name=f"khP{b}{c}")
                        nc.scalar.copy(khP[:], pk[:, :T])
                        qP = persistp.tile([128, U], f32, tag=f"qP{b}{c}",
                                           name=f"qP{b}{c}")
                        nc.scalar.copy(qP[:], pq[:, :U])
                        khPs[(b, c)], qPs[(b, c)] = khP, qP

                lp = [psL.tile([128, 512], f32, tag=f"lp{g}", name=f"lp{g}")
                      for g in range(4)]
                bi = 0
                for b in range(BL):
                    for c in range(2):
                        khP, qP = khPs[(b, c)], qPs[(b, c)]
                        bc_idx = 2 * b + c
                        if bc_idx == 0:
                            slab_plan = [2, 2, 4, 8]
                        elif bc_idx == 3:
                            slab_plan = [8, 4, 2, 2]
                        else:
                            slab_plan = [8, 8]
                        n0 = 0
                        for NS in slab_plan:
                            # base slab: base[d,(u,t)] = khP[d,t] + qP[d,u],
                            # built FD=128 at a time (u-specific bias), split
                            # between ScalarE and VectorE.
                            bsb = bsbp.tile([128, NS * 512], f16, tag="bsb",
                                            name="bsb")
                            for dn in range(NS):
                                for du in range(4):
                                    u = 4 * (n0 + dn) + du
                                    dst = bsb[:, dn * 512 + du * 128:
                                              dn * 512 + (du + 1) * 128]
                                    dve_build = (bi % 20 >= 13
                                                 if bc_idx < 3 else
                                                 bi % 20 >= 17)
                                    if dve_build:
                                        nc.vector.tensor_scalar(
                                            dst, khP[:], qP[:, u:u + 1], None,
                                            ALU.add)
                                    else:
                                        nc.scalar.activation(
                                            dst, khP[:], AF.Identity,
                                            bias=qP[:, u:u + 1], scale=1.0)
                                    bi += 1
                            for hp in range(2):
                                # heads hp and hp+2 land in different PE
                                # column groups (strips 2b, 2b+1): interleave
                                # their matmuls so the streams run
                                # concurrently in the array.
                                rts = {}
                                for h in (hp, hp + 2):
                                    rt = relup.tile([128, NS * 512], f16,
                                                    tag="rt", name="rt")
                                    bias_ap = A[:, _C_HQPB + c * 4 + h:
                                                _C_HQPB + c * 4 + h + 1]
                                    if bc_idx == 3 and n0 >= 12:
                                        nc.scalar.activation(
                                            rt[:], bsb[:], AF.Relu,
                                            bias=bias_ap, scale=1.0)
                                    else:
                                        nc.vector.tensor_scalar(
                                            rt[:], bsb[:], bias_ap, 0.0,
                                            ALU.add, ALU.max)
                                    rts[h] = rt
                                for dn in range(NS):
                                    n = n0 + dn
                                    for h in (hp, hp + 2):
                                        p_ = (b * H + h) * NBLK + n
                                        g, j = p_ // 32, p_ % 32
                                        first = (c == 0 and n == 0
                                                 and h % 2 == 0)
                                        last = (c == 1 and n == NBLK - 1
                                                and h % 2 == 1)
                                        nc.tensor.matmul(
                                            lp[g][32 * g:32 * g + 32, :],
                                            sb_wz[:, c * 63 + 31 - j:
                                                  c * 63 + 63 - j],
                                            rts[h][:, dn * 512:(dn + 1) * 512],
                                            start=first, stop=last,
                                            tile_position=(0, 32 * g))
                            n0 += NS

                sb_out = outp.tile([128, 512], f32, tag="sbout", name="sbout")
                for g in range(4):
                    nc.vector.tensor_scalar(
                        sb_out[32 * g:32 * g + 32, :],
                        lp[g][32 * g:32 * g + 32, :],
                        A[32 * g:32 * g + 32, _C_ENCB + 6:_C_ENCB + 7],
                        None, ALU.add)
                nc.sync.dma_start(out[:], sb_out[:])
    return nc


def _prep_inputs(uav_feat, task_feat, uw0, ub0, uw1, ub1, uw2, ub2,
                 tw0, tb0, tw1, tb1, tw2, tb2, head_q, fw1, fb1, fw2, fb2):
    f = np.float32
    uav = np.asarray(uav_feat, f)
    task = np.asarray(task_feat, f)
    fw1 = np.asarray(fw1, f)
    fw2 = np.asarray(fw2, f)
    Wq, Wk = fw1[:E], fw1[E:]

    base = np.zeros((128, _C_TOTAL), f)
    base[0:32, _C_UW0:_C_UW0 + 128] = np.asarray(uw0, f)
    base[0:32, _C_TW0:_C_TW0 + 128] = np.asarray(tw0, f)
    base[:, _C_UW1:_C_UW1 + 128] = np.asarray(uw1, f)
    base[:, _C_UW2:_C_UW2 + 128] = np.asarray(uw2, f)
    base[:, _C_TW1:_C_TW1 + 128] = np.asarray(tw1, f)
    base[:, _C_TW2:_C_TW2 + 128] = np.asarray(tw2, f)
    for i, v in enumerate((ub0, ub1, ub2, tb0, tb1, tb2)):
        base[:, _C_ENCB + i] = np.asarray(v, f)
    base[:, _C_ENCB + 6] = np.asarray(fb2, f)[0]
    hq = np.asarray(head_q, f) @ Wq + np.asarray(fb1, f)  # (H, HID)
    for c in range(2):
        for h in range(H):
            base[:, _C_HQPB + c * 4 + h] = hq[h, c * 128:(c + 1) * 128]
    base[:, _C_WQK:_C_WQK + 256] = Wq
    base[:, _C_WQK + 256:_C_WQK + 512] = Wk
    for c in range(2):
        base[:, _C_WZ + c * 63 + 31] = fw2[c * 128:(c + 1) * 128, 0]

    in_maps = []
    for k in range(NCORES):
        b0 = k * BL
        pk = base.copy()
        pk[0:32, _C_UAVT:_C_UAVT + BL * U] = \
            uav[b0:b0 + BL].reshape(BL * U, UAV_DIM).T
        pk[0:32, _C_TASKT:_C_TASKT + BL * T] = \
            task[b0:b0 + BL].reshape(BL * T, TASK_DIM).T
        in_maps.append({"packed": pk})
    return in_maps


def _gather(results):
    outs = []
    for k in range(NCORES):
        r = np.asarray(results[k]["out"], np.float32)  # (128, 512)
        outs.append(r.reshape(BL, H, NBLK, 4, T).reshape(BL, H, U, T))
    return np.concatenate(outs, axis=0)


def kernel(**inputs) -> np.ndarray:
    if "nc" not in _cache:
        _cache["nc"] = _build_nc()
    nc = _cache["nc"]
    in_maps = _prep_inputs(**inputs)
    if os.environ.get("BASS_KERNEL_SIM"):
        # CoreSim can't digest the hand-inserted wait-splitting NoOps; it
        # enforces the multi-wait semantics natively, so run unsplit.
        from concourse.bass_interp import CoreSim
        results = []
        for k in range(NCORES):
            sim = CoreSim(nc)
            for name, arr in in_maps[k].items():
                sim.tensor(name)[:] = arr
            sim.simulate()
            results.append({"out": np.array(sim.tensor("out"))})
    else:
        from concourse.bass_utils import run_bass_kernel_spmd
        if not _cache.get("split"):
            _split_multi_waits(nc)
            _cache["split"] = True
        results = run_bass_kernel_spmd(nc, in_maps, list(range(NCORES))).results
    return _gather(results)



# revision 3
# speedup vs baseline: 1.0600x; 1.0600x over previous
"""Trainium2 Bass kernel for nn_MultiHeadHighLevelAllocator.

Math (reference):
    ue = MLP3(uav_feat)                            # (B,U,E)
    te = MLP3(task_feat)                           # (B,T,E)
    q  = ue[:,None,:,:] + head_q[None,:,None,:]    # (B,H,U,E)
    logits[b,h,u,t] = relu(q[b,h,u]@Wq + te[b,t]@Wk + fb1) @ fw2 + fb2

Key decomposition: by linearity of the projections,
    pre[b,h,u,t,:] = base[b,u,t,:] + hqP[h,:]
where base[b,u,t,:] = ue[b,u]@Wq + te[b,t]@Wk  (outer sum, H-independent)
and   hqP[h,:] = head_q[h]@Wq + fb1.

Per-core (data parallel over B, 2 batches/core), per (b, c-chunk of HID):
  1. Encoders on TensorE in transposed layout, ScalarE ReLU+bias.
  2. khP = Wk te (128,T) and qp8 = Wq ue with each u-column repeated 8x
     (stride-0 moving AP on the projection matmul), evicted fp16.
  3. base slab (128, U*T) built by ONE DVE tensor_tensor in 2x_1p mode:
     in0 = khP broadcast over u (inner [t:1x16] runs keep mode), in1 = qp8
     re-read via [u:8x64][rep:0x16][r:1x8] (innermost stride-1 run of 8).
  4. Per head h: rt = relu(base + hqP[h]) as ONE big tensor_scalar
     (DVE 4x mode) or ScalarE activation, split to balance the engines.
  5. Reduction via masked-stationary matmuls: (128x32) fp16 stationary
     holding fw2-chunk in column j writes the dot product to PSUM
     partition 32g+j of strip g's bank; 4 strips run concurrently
     (round-robin tile_position col groups).
  6. One (128x512) fp32 result tile (+fb2) DMAed out per core.

All per-core inputs are packed host-side into a single (128, 1805) fp32
tensor loaded by phased DMAs (PE instructions only support a single sync
wait, so first-use deps must collapse to one semaphore).
"""
import os
import sys

for _p in ("/opt/trn_rl_repo", "/root/.axon_site/_ro/trn_rl_repo"):
    if os.path.isdir(_p) and _p not in sys.path:
        sys.path.insert(0, _p)

import numpy as np
import concourse.bass as bass
import concourse.mybir as mybir
from concourse import tile

B, U, T = 16, 64, 128
UAV_DIM, TASK_DIM = 32, 32
E, H, HID = 128, 4, 256
ENC_H = 128
NCORES = 8
BL = B // NCORES          # batches per core
NBLK = U // 4             # 16 u-blocks of 4 us -> N=512 columns each
f32, f16 = mybir.dt.float32, mybir.dt.float16
AF = mybir.ActivationFunctionType
ALU = mybir.AluOpType

# packed constant-tensor column layout (fp32 columns)
_C_UAVT = 0          # (32, 128)
_C_TASKT = 128       # (32, 256)
_C_UW0 = 384         # (32, 128)
_C_TW0 = 512         # (32, 128)
_C_UW1 = 640         # (128, 128)
_C_UW2 = 768
_C_TW1 = 896
_C_TW2 = 1024
_C_ENCB = 1152       # (128, 7): ub0 ub1 ub2 tb0 tb1 tb2 fb2
_C_HQPB = 1159       # (128, 8): col c*4+h
_C_WQK = 1167        # (128, 512): Wq c0 | Wq c1 | Wk c0 | Wk c1
# wz: two 63-col fp16 segments; fw2 chunk c at col c*63+31. The (128x32)
# masked stationary with fw2 at column j is the window [c*63+31-j, +32).
_C_WZ = 1679
_C_TOTAL = 1805

_cache: dict = {}


def _split_multi_waits(nc):
    """Walrus in this toolchain rejects >1 sync wait per engine instruction
    ("Too many sync wait commands"). Hoist extra waits onto preceding
    same-engine NoOps — identical semantics on the in-order engine queues."""
    n_split = 0
    for func in nc.m.functions:
        for bb in func.blocks:
            new = []
            for ins in bb.instructions:
                si = ins.sync_info
                waits = list(si.on_wait) if (si and si.on_wait) else []
                if len(waits) > 1:
                    for k, w in enumerate(waits[:-1]):
                        nop = mybir.InstNoOp(name=f"{ins.name}_hw{k}", ins=[], outs=[])
                        nop.engine = ins.engine
                        nop.sync_info = mybir.SyncInfo(on_wait=[w], on_update=[])
                        new.append(nop)
                        n_split += 1
                    si.on_wait = [waits[-1]]
                new.append(ins)
            bb.instructions = new
    return n_split


def _build_nc(rt_plan=None):
    # rt_plan: engine per (c, b, h) slab: 'D' (vector) or 'S' (scalar)
    if rt_plan is None:
        rt_plan = {}
        for c in range(2):
            for b in range(BL):
                for h in range(H):
                    # ScalarE takes one wave-A slab (b1,h2) and one wave-B
                    # slab (b1,h1) per section; DVE the other six.
                    rt_plan[(c, b, h)] = 'S' if (b == 1 and h in (1, 2)) else 'D'
    nc = bass.Bass()
    packed = nc.dram_tensor("packed", [128, _C_TOTAL], f32, kind="ExternalInput")
    out = nc.dram_tensor("out", [128, 512], f32, kind="ExternalOutput")

    with tile.TileContext(nc) as tc:
        with (
            tc.tile_pool(name="const", bufs=1) as constp,
            tc.tile_pool(name="persist", bufs=1) as persistp,
            tc.tile_pool(name="encw", bufs=2) as encwp,
        ):
            A = constp.tile([128, _C_TOTAL], f32, tag="all")
            # phase-ordered loads so the encoders start as early as possible
            nc.sync.dma_start(A[:, :640], packed[:, :640])           # L1 inputs
            nc.sync.dma_start(A[:, 1152:_C_WQK], packed[:, 1152:_C_WQK])  # biases
            nc.sync.dma_start(A[:, 640:1152], packed[:, 640:1152])   # L2/L3 w
            nc.sync.dma_start(A[:, _C_WQK:], packed[:, _C_WQK:])     # wqk+wz
            # first touches per engine so later ops never pair a DMA-sem wait
            # with an engine-sem wait in one instruction
            act_touch = constp.tile([128, 1], f32, tag="acttouch")
            nc.scalar.copy(act_touch[:], A[:, 0:1])
            dve_touch = constp.tile([128, 1], f32, tag="dvetouch")
            nc.vector.tensor_copy(dve_touch[:], A[:, 0:1])
            # fp16 projection weights: fp32 moving operands stream at half
            # rate through the PE array, so projections run fp16.
            sb_wqk16 = constp.tile([128, 512], f16, tag="wqk16")
            nc.scalar.copy(sb_wqk16[:], A[:, _C_WQK:_C_WQK + 512])
            # wz holds fw2 chunk c in column c*63+31; the (128x32) masked
            # stationary with fw2 at column j is the window [c*63+31-j, +32).
            sb_wz = constp.tile([128, 126], f16, tag="wz")
            nc.scalar.copy(sb_wz[:], A[:, _C_WZ:_C_WZ + 126])

            enc_w = {
                "uw0": A[0:32, _C_UW0:_C_UW0 + 128],
                "tw0": A[0:32, _C_TW0:_C_TW0 + 128],
                "uw1": A[:, _C_UW1:_C_UW1 + 128],
                "uw2": A[:, _C_UW2:_C_UW2 + 128],
                "tw1": A[:, _C_TW1:_C_TW1 + 128],
                "tw2": A[:, _C_TW2:_C_TW2 + 128],
            }

            def encb_col(i):
                return A[:, _C_ENCB + i:_C_ENCB + i + 1]

            with (
                tc.tile_pool(name="bsbp", bufs=4) as bsbp,
                tc.tile_pool(name="relup", bufs=6) as relup,
                tc.tile_pool(name="outp", bufs=1) as outp,
                tc.tile_pool(name="bpp", bufs=4, space="PSUM") as psB,
                tc.tile_pool(name="lpp", bufs=1, space="PSUM") as psL,
            ):
                # ---- encoders: ue/te chains interleaved so PE and
                #      ScalarE ping-pong instead of serializing ----
                chains = {
                    "ue": [A[0:32, _C_UAVT:_C_UAVT + BL * U], BL * U,
                           ("uw0", "uw1", "uw2"), (0, 1, 2)],
                    "te": [A[0:32, _C_TASKT:_C_TASKT + BL * T], BL * T,
                           ("tw0", "tw1", "tw2"), (3, 4, 5)],
                }
                cur = {k: v[0] for k, v in chains.items()}
                for li in range(3):
                    pss = {}
                    for k, (x0, rows, wn, bc) in chains.items():
                        ps = psB.tile([128, 512], f32, tag="bp",
                                      name=f"ps{k}{li}")
                        nc.tensor.matmul(ps[:, :rows], enc_w[wn[li]], cur[k],
                                         start=True, stop=True)
                        pss[k] = ps
                    for k, (x0, rows, wn, bc) in chains.items():
                        if li < 2:
                            nxt = encwp.tile([128, rows], f32, tag=f"{k}h",
                                             name=f"{k}h{li}")
                            nc.scalar.activation(nxt[:], pss[k][:, :rows],
                                                 AF.Relu,
                                                 bias=encb_col(bc[li]),
                                                 scale=1.0)
                        else:
                            nxt = persistp.tile([128, rows], f16, tag=f"{k}T",
                                                name=f"{k}T")
                            nc.scalar.activation(nxt[:], pss[k][:, :rows],
                                                 AF.Identity,
                                                 bias=encb_col(bc[li]),
                                                 scale=1.0)
                        cur[k] = nxt[:]
                ueT, teT = cur["ue"], cur["te"]

                # ---- projections for all (b,c): khP (128,T) f16 and
                #      qp8 (128,512) f16 = Wq ue with u-cols repeated 8x ----
                khPs, qp8s = {}, {}
                for c in range(2):
                    for b in range(BL):
                        pk = psB.tile([128, 512], f32, tag="bp",
                                      name=f"pk{b}{c}")
                        nc.tensor.matmul(pk[:, :T],
                                         sb_wqk16[:, 256 + c * 128:256 + (c + 1) * 128],
                                         teT[:, b * T:(b + 1) * T],
                                         start=True, stop=True)
                        khP = persistp.tile([128, T], f16, tag=f"khP{b}{c}",
                                            name=f"khP{b}{c}")
                        nc.scalar.copy(khP[:], pk[:, :T])
                        pq = psB.tile([128, 512], f32, tag="bp",
                                      name=f"pq{b}{c}")
                        mov = ueT[:, b * U:(b + 1) * U].unsqueeze(2) \
                            .to_broadcast([128, U, 8])
                        nc.tensor.matmul(pq[:].rearrange("p (u r) -> p u r", r=8),
                                         sb_wqk16[:, c * 128:(c + 1) * 128],
                                         mov, start=True, stop=True)
                        qp8 = persistp.tile([128, 512], f16, tag=f"qp8{b}{c}",
                                            name=f"qp8{b}{c}")
                        nc.scalar.copy(qp8[:], pq[:])
                        khPs[(b, c)], qp8s[(b, c)] = khP, qp8

                # strip for (b,h): wave A h in (0,2), wave B h in (1,3)
                # p = (b*H+h)*NBLK + n; g = p // 32; j = p % 32
                lp = [psL.tile([128, 512], f32, tag=f"lp{g}", name=f"lp{g}")
                      for g in range(4)]

                def emit_rt(c, b, h, bsb):
                    eng = rt_plan[(c, b, h)]
                    rt = relup.tile([128, U * T], f16, tag="rt",
                                    name=f"rt{c}{b}{h}")
                    bias_ap = A[:, _C_HQPB + c * 4 + h:_C_HQPB + c * 4 + h + 1]
                    if eng == 'S':
                        nc.scalar.activation(rt[:], bsb[:], AF.Relu,
                                             bias=bias_ap, scale=1.0)
                    else:
                        nc.vector.tensor_scalar(rt[:], bsb[:], bias_ap, 0.0,
                                                ALU.add, ALU.max)
                    return rt

                def emit_wave(c, wave_slabs):
                    # wave_slabs: list of (b, h, rt) covering 4 distinct strips
                    for n in range(NBLK):
                        for (b, h, rt) in wave_slabs:
                            p_ = (b * H + h) * NBLK + n
                            g, j = p_ // 32, p_ % 32
                            first = (c == 0 and n == 0 and h % 2 == 0)
                            last = (c == 1 and n == NBLK - 1 and h % 2 == 1)
                            nc.tensor.matmul(
                                lp[g][32 * g:32 * g + 32, :],
                                sb_wz[:, c * 63 + 31 - j:c * 63 + 63 - j],
                                rt[:, n * 512:(n + 1) * 512],
                                start=first, stop=last,
                                tile_position=(0, 32 * g))

                for c in range(2):
                    bsbs = {}
                    for b in range(BL):
                        bsb = bsbp.tile([128, U * T], f16, tag="bsb",
                                        name=f"bsb{b}{c}")
                        khP, qp8 = khPs[(b, c)], qp8s[(b, c)]
                        dst = bsb[:].rearrange("p (u v r) -> p u v r",
                                               v=16, r=8)
                        in0 = khP[:].rearrange("p (v r) -> p v r", r=8) \
                            .unsqueeze(1).to_broadcast([128, U, 16, 8])
                        in1 = qp8[:].rearrange("p (u r) -> p u r", r=8) \
                            .unsqueeze(2).to_broadcast([128, U, 16, 8])
                        nc.vector.tensor_tensor(dst, in0, in1, ALU.add)
                        bsbs[b] = bsb
                    # wave A: h in (0,2) for both b -> strips 2b + h/2
                    waveA = []
                    for h in (0, 2):
                        for b in range(BL):
                            waveA.append((b, h, emit_rt(c, b, h, bsbs[b])))
                    emit_wave(c, waveA)
                    waveB = []
                    for h in (1, 3):
                        for b in range(BL):
                            waveB.append((b, h, emit_rt(c, b, h, bsbs[b])))
                    emit_wave(c, waveB)

                sb_out = outp.tile([128, 512], f32, tag="sbout", name="sbout")
                for g in range(4):
                    nc.scalar.activation(
                        sb_out[32 * g:32 * g + 32, :],
                        lp[g][32 * g:32 * g + 32, :],
                        AF.Identity,
                        bias=A[32 * g:32 * g + 32, _C_ENCB + 6:_C_ENCB + 7],
                        scale=1.0)
                nc.sync.dma_start(out[:], sb_out[:])
    return nc


def _prep_inputs(uav_feat, task_feat, uw0, ub0, uw1, ub1, uw2, ub2,
                 tw0, tb0, tw1, tb1, tw2, tb2, head_q, fw1, fb1, fw2, fb2):
    f = np.float32
    uav = np.asarray(uav_feat, f)
    task = np.asarray(task_feat, f)
    fw1 = np.asarray(fw1, f)
    fw2 = np.asarray(fw2, f)
    Wq, Wk = fw1[:E], fw1[E:]

    base = np.zeros((128, _C_TOTAL), f)
    base[0:32, _C_UW0:_C_UW0 + 128] = np.asarray(uw0, f)
    base[0:32, _C_TW0:_C_TW0 + 128] = np.asarray(tw0, f)
    base[:, _C_UW1:_C_UW1 + 128] = np.asarray(uw1, f)
    base[:, _C_UW2:_C_UW2 + 128] = np.asarray(uw2, f)
    base[:, _C_TW1:_C_TW1 + 128] = np.asarray(tw1, f)
    base[:, _C_TW2:_C_TW2 + 128] = np.asarray(tw2, f)
    for i, v in enumerate((ub0, ub1, ub2, tb0, tb1, tb2)):
        base[:, _C_ENCB + i] = np.asarray(v, f)
    base[:, _C_ENCB + 6] = np.asarray(fb2, f)[0]
    hq = np.asarray(head_q, f) @ Wq + np.asarray(fb1, f)  # (H, HID)
    for c in range(2):
        for h in range(H):
            base[:, _C_HQPB + c * 4 + h] = hq[h, c * 128:(c + 1) * 128]
    base[:, _C_WQK:_C_WQK + 256] = Wq
    base[:, _C_WQK + 256:_C_WQK + 512] = Wk
    for c in range(2):
        base[:, _C_WZ + c * 63 + 31] = fw2[c * 128:(c + 1) * 128, 0]

    in_maps = []
    for k in range(NCORES):
        b0 = k * BL
        pk = base.copy()
        pk[0:32, _C_UAVT:_C_UAVT + BL * U] = \
            uav[b0:b0 + BL].reshape(BL * U, UAV_DIM).T
        pk[0:32, _C_TASKT:_C_TASKT + BL * T] = \
            task[b0:b0 + BL].reshape(BL * T, TASK_DIM).T
        in_maps.append({"packed": pk})
    return in_maps


def _gather(results):
    outs = []
    for k in range(NCORES):
        r = np.asarray(results[k]["out"], np.float32)  # (128, 512)
        outs.append(r.reshape(BL, H, NBLK, 4, T).reshape(BL, H, U, T))
    return np.concatenate(outs, axis=0)


def kernel(**inputs) -> np.ndarray:
    if "nc" not in _cache:
        _cache["nc"] = _build_nc()
    nc = _cache["nc"]
    in_maps = _prep_inputs(**inputs)
    if os.environ.get("BASS_KERNEL_SIM"):
        # CoreSim can't digest the hand-inserted wait-splitting NoOps; it
        # enforces the multi-wait semantics natively, so run unsplit.
        from concourse.bass_interp import CoreSim
        results = []
        for k in range(NCORES):
            sim = CoreSim(nc)
            for name, arr in in_maps[k].items():
                sim.tensor(name)[:] = arr
            sim.simulate()
            results.append({"out": np.array(sim.tensor("out"))})
    else:
        from concourse.bass_utils import run_bass_kernel_spmd
        if not _cache.get("split"):
            _split_multi_waits(nc)
            _cache["split"] = True
        results = run_bass_kernel_spmd(nc, in_maps, list(range(NCORES))).results
    return _gather(results)


# revision 4
# speedup vs baseline: 1.2471x; 1.1764x over previous
"""Trainium2 Bass kernel for nn_MultiHeadHighLevelAllocator.

Math (reference):
    ue = MLP3(uav_feat)                            # (B,U,E)
    te = MLP3(task_feat)                           # (B,T,E)
    q  = ue[:,None,:,:] + head_q[None,:,None,:]    # (B,H,U,E)
    logits[b,h,u,t] = relu(q[b,h,u]@Wq + te[b,t]@Wk + fb1) @ fw2 + fb2

Key decomposition: by linearity of the projections,
    pre[b,h,u,t,:] = base[b,u,t,:] + hqP[h,:]
where base[b,u,t,:] = ue[b,u]@Wq + te[b,t]@Wk  (outer sum, H-independent)
and   hqP[h,:] = head_q[h]@Wq + fb1.

Per-core (data parallel over B, 2 batches/core), per (b, c-chunk of HID):
  1. Encoders on TensorE; ReLU+bias on DVE (ue chain) / ScalarE (te chain).
  2. khP = Wk te (128,T) and qp8 = Wq ue with each u-column repeated 8x
     (stride-0 moving AP on the projection matmul), evicted fp16.
  3. base slab (128, U*T) built by ONE DVE tensor_tensor in 2x_1p mode:
     in0 = khP broadcast over u (inner stride-1 runs of 8 keep the mode),
     in1 = qp8 re-read via [u:8x64][rep:0x16][r:1x8].
  4. Per head h: rt = relu(base + hqP[h]) as ONE big tensor_scalar
     (DVE 4x mode) or ScalarE activation; the split is tuned so ScalarE's
     slower slabs are scheduled early and never gate the matmul waves.
  5. Reduction via masked-stationary matmuls: (128x32) fp16 stationary
     holding fw2-chunk in column j writes the dot product to PSUM
     partition 32g+j of strip g's bank; subwaves of 2 strips issue
     round-robin so up to 4 col-strips stream concurrently.
  6. One (128x512) fp32 result tile (+fb2) DMAed out per core.

Inputs are packed host-side into one fp16 tensor (weights/activations)
plus a small fp32 tensor (biases), loaded by phased DMAs.
"""
import os
import sys

for _p in ("/opt/trn_rl_repo", "/root/.axon_site/_ro/trn_rl_repo"):
    if os.path.isdir(_p) and _p not in sys.path:
        sys.path.insert(0, _p)

import numpy as np
import concourse.bass as bass
import concourse.mybir as mybir
from concourse import tile

B, U, T = 16, 64, 128
UAV_DIM, TASK_DIM = 32, 32
E, H, HID = 128, 4, 256
ENC_H = 128
NCORES = 8
BL = B // NCORES          # batches per core
NBLK = U // 4             # 16 u-blocks of 4 us -> N=512 columns each
f32, f16 = mybir.dt.float32, mybir.dt.float16
AF = mybir.ActivationFunctionType
ALU = mybir.AluOpType

# fp16 packed tensor column layout
_F_UAVT = 0          # (32, 128)
_F_TASKT = 128       # (32, 256)
_F_UW0 = 384         # (32, 128)
_F_TW0 = 512         # (32, 128)
_F_UW1 = 640         # (128, 128)
_F_UW2 = 768
_F_TW1 = 896
_F_TW2 = 1024
_F_WQK = 1152        # (128, 512): Wq c0 | Wq c1 | Wk c0 | Wk c1
# wz holds fw2 chunk c in column c*63+31; the (128x32) masked stationary
# with fw2 at column j is the window [c*63+31-j, +32).
_F_WZ = 1664         # (128, 126)
_F_TOT = 1790
# fp32 packed tensor: 7 encoder biases + fb2, then 8 hqP columns
_G_ENCB = 0          # ub0 ub1 ub2 tb0 tb1 tb2 fb2
_G_HQPB = 7          # col c*4+h
_G_TOT = 15

_cache: dict = {}


def _split_multi_waits(nc):
    """Walrus in this toolchain rejects >1 sync wait per engine instruction
    ("Too many sync wait commands"). Hoist extra waits onto preceding
    same-engine NoOps — identical semantics on the in-order engine queues."""
    n_split = 0
    for func in nc.m.functions:
        for bb in func.blocks:
            new = []
            for ins in bb.instructions:
                si = ins.sync_info
                waits = list(si.on_wait) if (si and si.on_wait) else []
                if len(waits) > 1:
                    for k, w in enumerate(waits[:-1]):
                        nop = mybir.InstNoOp(name=f"{ins.name}_hw{k}", ins=[], outs=[])
                        nop.engine = ins.engine
                        nop.sync_info = mybir.SyncInfo(on_wait=[w], on_update=[])
                        new.append(nop)
                        n_split += 1
                    si.on_wait = [waits[-1]]
                new.append(ins)
            bb.instructions = new
    return n_split


def _build_nc():
    nc = bass.Bass()
    p16 = nc.dram_tensor("p16", [128, _F_TOT], f16, kind="ExternalInput")
    p32 = nc.dram_tensor("p32", [128, _G_TOT], f32, kind="ExternalInput")
    out = nc.dram_tensor("out", [128, 512], f32, kind="ExternalOutput")

    with tile.TileContext(nc) as tc:
        with (
            tc.tile_pool(name="const", bufs=1) as constp,
            tc.tile_pool(name="persist", bufs=1) as persistp,
            tc.tile_pool(name="encw", bufs=2) as encwp,
        ):
            A = constp.tile([128, _F_TOT], f16, tag="a16")
            G = constp.tile([128, _G_TOT], f32, tag="a32")
            # phase-ordered loads so the encoders start as early as possible
            nc.sync.dma_start(A[:, :640], p16[:, :640])        # inputs + l1 w
            nc.sync.dma_start(G[:], p32[:])                    # biases
            nc.sync.dma_start(A[:, 640:_F_WQK], p16[:, 640:_F_WQK])  # l2/l3 w
            nc.sync.dma_start(A[:, _F_WQK:], p16[:, _F_WQK:])  # wqk + wz
            # first touches per engine so later ops never pair a DMA-sem wait
            # with an engine-sem wait in one instruction
            act_touch = constp.tile([128, 1], f32, tag="acttouch")
            nc.scalar.copy(act_touch[:], G[:, 0:1])
            dve_touch = constp.tile([128, 1], f32, tag="dvetouch")
            nc.vector.tensor_copy(dve_touch[:], G[:, 0:1])
            act_touch2 = constp.tile([128, 1], f16, tag="acttouch2")
            nc.scalar.copy(act_touch2[:], A[:, 0:1])
            dve_touch2 = constp.tile([128, 1], f16, tag="dvetouch2")
            nc.vector.tensor_copy(dve_touch2[:], A[:, 0:1])

            enc_w = {
                "uw0": A[0:32, _F_UW0:_F_UW0 + 128],
                "tw0": A[0:32, _F_TW0:_F_TW0 + 128],
                "uw1": A[:, _F_UW1:_F_UW1 + 128],
                "uw2": A[:, _F_UW2:_F_UW2 + 128],
                "tw1": A[:, _F_TW1:_F_TW1 + 128],
                "tw2": A[:, _F_TW2:_F_TW2 + 128],
            }

            def encb_col(i):
                return G[:, _G_ENCB + i:_G_ENCB + i + 1]

            def hqp_col(c, h):
                i = _G_HQPB + c * 4 + h
                return G[:, i:i + 1]

            with (
                tc.tile_pool(name="bsbp", bufs=4) as bsbp,
                tc.tile_pool(name="relup", bufs=6) as relup,
                tc.tile_pool(name="outp", bufs=1) as outp,
                tc.tile_pool(name="bpp", bufs=4, space="PSUM") as psB,
                tc.tile_pool(name="lpp", bufs=1, space="PSUM") as psL,
            ):
                # ---- encoders: ue acts on DVE, te acts on ScalarE so the
                #      two chains run in parallel ----
                chains = {
                    "ue": [A[0:32, _F_UAVT:_F_UAVT + BL * U], BL * U,
                           ("uw0", "uw1", "uw2"), (0, 1, 2)],
                    "te": [A[0:32, _F_TASKT:_F_TASKT + BL * T], BL * T,
                           ("tw0", "tw1", "tw2"), (3, 4, 5)],
                }
                cur = {k: v[0] for k, v in chains.items()}
                for li in range(3):
                    pss = {}
                    for k, (x0, rows, wn, bc) in chains.items():
                        ps = psB.tile([128, 512], f32, tag="bp",
                                      name=f"ps{k}{li}")
                        nc.tensor.matmul(ps[:, :rows], enc_w[wn[li]], cur[k],
                                         start=True, stop=True)
                        pss[k] = ps
                    for k, (x0, rows, wn, bc) in chains.items():
                        if li < 2:
                            nxt = encwp.tile([128, rows], f16, tag=f"{k}h",
                                             name=f"{k}h{li}")
                        else:
                            nxt = persistp.tile([128, rows], f16, tag=f"{k}T",
                                                name=f"{k}T")
                        if k == "ue":
                            if li < 2:
                                nc.vector.tensor_scalar(
                                    nxt[:], pss[k][:, :rows],
                                    encb_col(bc[li]), 0.0, ALU.add, ALU.max)
                            else:
                                nc.vector.tensor_scalar(
                                    nxt[:], pss[k][:, :rows],
                                    encb_col(bc[li]), None, ALU.add)
                        else:
                            nc.scalar.activation(
                                nxt[:], pss[k][:, :rows],
                                AF.Relu if li < 2 else AF.Identity,
                                bias=encb_col(bc[li]), scale=1.0)
                        cur[k] = nxt[:]
                ueT, teT = cur["ue"], cur["te"]

                # ---- projections for all (b,c): khP (128,T) f16 and
                #      qp8 (128,512) f16 = Wq ue with u-cols repeated 8x;
                #      (c0,b0) eviction on DVE (feeds its own first TT),
                #      the rest on ScalarE ----
                khPs, qp8s = {}, {}
                for c in range(2):
                    for b in range(BL):
                        pk = psB.tile([128, 512], f32, tag="bp",
                                      name=f"pk{b}{c}")
                        nc.tensor.matmul(pk[:, :T],
                                         A[:, _F_WQK + 256 + c * 128:
                                           _F_WQK + 256 + (c + 1) * 128],
                                         teT[:, b * T:(b + 1) * T],
                                         start=True, stop=True)
                        pq = psB.tile([128, 512], f32, tag="bp",
                                      name=f"pq{b}{c}")
                        mov = ueT[:, b * U:(b + 1) * U].unsqueeze(2) \
                            .to_broadcast([128, U, 8])
                        nc.tensor.matmul(pq[:].rearrange("p (u r) -> p u r", r=8),
                                         A[:, _F_WQK + c * 128:
                                           _F_WQK + (c + 1) * 128],
                                         mov, start=True, stop=True)
                        khP = persistp.tile([128, T], f16, tag=f"khP{b}{c}",
                                            name=f"khP{b}{c}")
                        qp8 = persistp.tile([128, 512], f16, tag=f"qp8{b}{c}",
                                            name=f"qp8{b}{c}")
                        if c == 0 and b == 0:
                            nc.vector.tensor_copy(khP[:], pk[:, :T])
                            nc.vector.tensor_copy(qp8[:], pq[:])
                        else:
                            nc.scalar.copy(khP[:], pk[:, :T])
                            nc.scalar.copy(qp8[:], pq[:])
                        khPs[(b, c)], qp8s[(b, c)] = khP, qp8

                lp = [psL.tile([128, 512], f32, tag=f"lp{g}", name=f"lp{g}")
                      for g in range(4)]

                def emit_build(c, b):
                    bsb = bsbp.tile([128, U * T], f16, tag="bsb",
                                    name=f"bsb{b}{c}")
                    khP, qp8 = khPs[(b, c)], qp8s[(b, c)]
                    dst = bsb[:].rearrange("p (u v r) -> p u v r", v=16, r=8)
                    in0 = khP[:].rearrange("p (v r) -> p v r", r=8) \
                        .unsqueeze(1).to_broadcast([128, U, 16, 8])
                    in1 = qp8[:].rearrange("p (u r) -> p u r", r=8) \
                        .unsqueeze(2).to_broadcast([128, U, 16, 8])
                    nc.vector.tensor_tensor(dst, in0, in1, ALU.add)
                    return bsb

                def emit_rt(c, b, h, bsb, eng, lo=0, hi=U * T):
                    rt = relup.tile([128, hi - lo], f16, tag="rt",
                                    name=f"rt{c}{b}{h}_{lo}")
                    if eng == 'S':
                        nc.scalar.activation(rt[:], bsb[:, lo:hi], AF.Relu,
                                             bias=hqp_col(c, h), scale=1.0)
                    else:
                        nc.vector.tensor_scalar(rt[:], bsb[:, lo:hi],
                                                hqp_col(c, h), 0.0,
                                                ALU.add, ALU.max)
                    return rt

                def emit_subwave(c, pair):
                    # pair: list of (b, h, [(n_lo, n_hi, rt_tile), ...])
                    for n in range(NBLK):
                        for (b, h, pieces) in pair:
                            rt = None
                            for (n_lo, n_hi, t_) in pieces:
                                if n_lo <= n < n_hi:
                                    rt, off = t_, n - n_lo
                                    break
                            p_ = (b * H + h) * NBLK + n
                            g, j = p_ // 32, p_ % 32
                            first = (c == 0 and n == 0 and h % 2 == 0)
                            last = (c == 1 and n == NBLK - 1 and h % 2 == 1)
                            nc.tensor.matmul(
                                lp[g][32 * g:32 * g + 32, :],
                                A[:, _F_WZ + c * 63 + 31 - j:
                                  _F_WZ + c * 63 + 63 - j],
                                rt[:, off * 512:(off + 1) * 512],
                                start=first, stop=last,
                                tile_position=(0, 32 * g))

                def whole(rt):
                    return [(0, NBLK, rt)]

                for c in range(2):
                    bsb0 = emit_build(c, 0)
                    # ScalarE starts its first (slow) slab immediately while
                    # DVE builds b1's base
                    rt_s1 = emit_rt(c, 0, 0, bsb0, 'S')
                    bsb1 = emit_build(c, 1)
                    rt_d1 = emit_rt(c, 1, 0, bsb1, 'D')
                    emit_subwave(c, [(0, 0, whole(rt_s1)),
                                     (1, 0, whole(rt_d1))])
                    rt_d2 = emit_rt(c, 0, 2, bsb0, 'D')
                    rt_d3 = emit_rt(c, 1, 2, bsb1, 'D')
                    emit_subwave(c, [(0, 2, whole(rt_d2)),
                                     (1, 2, whole(rt_d3))])
                    rt_s2 = emit_rt(c, 0, 1, bsb0, 'S')
                    rt_d4 = emit_rt(c, 1, 1, bsb1, 'D')
                    emit_subwave(c, [(0, 1, whole(rt_s2)),
                                     (1, 1, whole(rt_d4))])
                    # last subwave: (b0,h3) split ScalarE(first half)/DVE
                    rt_s3 = emit_rt(c, 0, 3, bsb0, 'S', 0, 8 * 512)
                    rt_d5 = emit_rt(c, 0, 3, bsb0, 'D', 8 * 512, U * T)
                    rt_d6 = emit_rt(c, 1, 3, bsb1, 'D')
                    emit_subwave(c, [(0, 3, [(0, 8, rt_s3), (8, NBLK, rt_d5)]),
                                     (1, 3, whole(rt_d6))])

                sb_out = outp.tile([128, 512], f32, tag="sbout", name="sbout")
                for g in (0, 2, 1, 3):
                    nc.scalar.activation(
                        sb_out[32 * g:32 * g + 32, :],
                        lp[g][32 * g:32 * g + 32, :],
                        AF.Identity,
                        bias=G[32 * g:32 * g + 32, _G_ENCB + 6:_G_ENCB + 7],
                        scale=1.0)
                nc.sync.dma_start(out[:], sb_out[:])
    return nc


def _prep_inputs(uav_feat, task_feat, uw0, ub0, uw1, ub1, uw2, ub2,
                 tw0, tb0, tw1, tb1, tw2, tb2, head_q, fw1, fb1, fw2, fb2):
    f, f16n = np.float32, np.float16
    uav = np.asarray(uav_feat, f)
    task = np.asarray(task_feat, f)
    fw1 = np.asarray(fw1, f)
    fw2 = np.asarray(fw2, f)
    Wq, Wk = fw1[:E], fw1[E:]

    b16 = np.zeros((128, _F_TOT), f16n)
    b16[0:32, _F_UW0:_F_UW0 + 128] = np.asarray(uw0, f16n)
    b16[0:32, _F_TW0:_F_TW0 + 128] = np.asarray(tw0, f16n)
    b16[:, _F_UW1:_F_UW1 + 128] = np.asarray(uw1, f16n)
    b16[:, _F_UW2:_F_UW2 + 128] = np.asarray(uw2, f16n)
    b16[:, _F_TW1:_F_TW1 + 128] = np.asarray(tw1, f16n)
    b16[:, _F_TW2:_F_TW2 + 128] = np.asarray(tw2, f16n)
    b16[:, _F_WQK:_F_WQK + 256] = Wq.astype(f16n)
    b16[:, _F_WQK + 256:_F_WQK + 512] = Wk.astype(f16n)
    for c in range(2):
        b16[:, _F_WZ + c * 63 + 31] = fw2[c * 128:(c + 1) * 128, 0].astype(f16n)

    b32 = np.zeros((128, _G_TOT), f)
    for i, v in enumerate((ub0, ub1, ub2, tb0, tb1, tb2)):
        b32[:, _G_ENCB + i] = np.asarray(v, f)
    b32[:, _G_ENCB + 6] = np.asarray(fb2, f)[0]
    hq = np.asarray(head_q, f) @ Wq + np.asarray(fb1, f)  # (H, HID)
    for c in range(2):
        for h in range(H):
            b32[:, _G_HQPB + c * 4 + h] = hq[h, c * 128:(c + 1) * 128]

    in_maps = []
    for k in range(NCORES):
        b0 = k * BL
        pk = b16.copy()
        pk[0:32, _F_UAVT:_F_UAVT + BL * U] = \
            uav[b0:b0 + BL].reshape(BL * U, UAV_DIM).T.astype(f16n)
        pk[0:32, _F_TASKT:_F_TASKT + BL * T] = \
            task[b0:b0 + BL].reshape(BL * T, TASK_DIM).T.astype(f16n)
        in_maps.append({"p16": pk, "p32": b32})
    return in_maps


def _gather(results):
    outs = []
    for k in range(NCORES):
        r = np.asarray(results[k]["out"], np.float32)  # (128, 512)
        outs.append(r.reshape(BL, H, NBLK, 4, T).reshape(BL, H, U, T))
    return np.concatenate(outs, axis=0)


def kernel(**inputs) -> np.ndarray:
    if "nc" not in _cache:
        _cache["nc"] = _build_nc()
    nc = _cache["nc"]
    in_maps = _prep_inputs(**inputs)
    if os.environ.get("BASS_KERNEL_SIM"):
        # CoreSim can't digest the hand-inserted wait-splitting NoOps; it
        # enforces the multi-wait semantics natively, so run unsplit.
        from concourse.bass_interp import CoreSim
        results = []
        for k in range(NCORES):
            sim = CoreSim(nc)
            for name, arr in in_maps[k].items():
                sim.tensor(name)[:] = arr
            sim.simulate()
            results.append({"out": np.array(sim.tensor("out"))})
    else:
        from concourse.bass_utils import run_bass_kernel_spmd
        if not _cache.get("split"):
            _split_multi_waits(nc)
            _cache["split"] = True
        results = run_bass_kernel_spmd(nc, in_maps, list(range(NCORES))).results
    return _gather(results)
